# revision 55
# baseline (speedup 1.0000x reference)
"""Trainium2 Bass kernel for nn_ExpandEvecs.

Computes, for evecs [B=4, C=1, N=1024, K=16]:
    cube[b,l] = V[:, :l+1] @ V[:, :l+1]^T   (Gram expansion per level)
    -> [B, K, N, N] fp32 (cumsum of per-eigvec outer products over l).

Sharding: 8 cores = 4 batches x 2 row-halves; core c (b=c//2, h=c%2)
produces all 16 levels for its 512-row half. No communication.

Performance model (per core, 8.4M output elements; all rates HW-measured):
  - The PE streams one 512-column matmul per 427 ns (1.2 GHz sustained;
    the 2.4 GHz p-state needs 3 us of gapless execution, unreachable
    when PSUM drain paces the PE) -> 128 matmuls = 54.7 us. This is the
    kernel's floor: the PE is the only engine that can produce outer
    products at rate (GpSimd tensor ops measured 2.1-15 us per 131K
    elems, DVE fused STT 9.4 us -> offload designs all lose).
  - PSUM evacuation: only ACT (1.2 GHz) and DVE (0.96 GHz) have PSUM
    ports. Whole [128, 2048] ops (one 4-bank PSUM tile) alternate
    between them ~53:47 -> ~35 us in parallel, under the PE floor.
  - int8 output (8.4 MB -> ~23.5 us at the ~358 GB/s per-core HBM
    limit) keeps DMA far off the critical path; fp32 would be 94 us.

Precision (gate 2e-2; simulated end-to-end 4.5e-3):
  - fp8 split matmuls: V = A + B, A = e4m3(V), B = e4m3(V-A);
    V V^T ~= A A^T + A B^T + B A^T (dropped B B^T ~2^-8). 4 stacked
    rows per eigvec k (lhs A,B,A,0 / rhs A,A,B,0) so DoubleRow pairs
    never straddle a k boundary.
  - int8 scale per (level, partition) from the Cauchy-Schwarz bound
    max over the partition's 4 interleaved rows of
    ||v_i||_l * max_j ||v_j||_l (host-computed, 2% margin), applied
    during evacuation (ACT activation scale= / DVE tensor_scalar,
    which round to nearest). Host dequantizes during the unshard.
  - Row-pair interleave: partition p holds DRAM rows 4p..4p+3, giving
    4 KiB contiguous int8 store runs per partition.
"""

import numpy as np
import ml_dtypes

import concourse.mybir as mybir
from concourse import bacc, bass
from concourse.tile import TileContext
from concourse.bass_utils import run_bass_kernel_spmd

B, C, N, K = 4, 1, 1024, 16
NCORES = 8
HALF = N // 2          # rows per core
KI = 2 * K             # DoubleRow pair-partitions at the deepest level

F32 = mybir.dt.float32
FP16 = mybir.dt.float16
FP8 = mybir.dt.float8e4
I8 = mybir.dt.int8
FP8_NP = ml_dtypes.float8_e4m3

_nc_cache = None


def _build():
    nc = bacc.Bacc(None, target_bir_lowering=False)
    t_d = nc.declare_dram_parameter("t", [KI, 2, N], FP8, isOutput=False)
    tl_d = nc.declare_dram_parameter("tl", [KI, 2, HALF], FP8, isOutput=False)
    # sc columns [0, 4K) = per-(level,row) int8 scales; [4K, 4K+4) =
    # v0*s0 for the rank-1 direct level 0; [4K+4, 4K+8) = v0*s1 and
    # [4K+8, 4K+12) = v1*s1 for the rank-2 direct level 1
    sc_d = nc.declare_dram_parameter("sc", [128, K * 4 + 12], F32, isOutput=False)
    out_d = nc.declare_dram_parameter("out", [K, HALF, N], I8, isOutput=True)
    # level 1 ships as unscaled fp16: its rank-2 direct path adds on
    # GpSimd, whose tensor_tensor only codegens with fp16 output
    o1_d = nc.declare_dram_parameter("o1", [HALF, N], FP16, isOutput=True)

    DR = mybir.MatmulPerfMode.DoubleRow
    COPY = mybir.ActivationFunctionType.Copy
    MUL = mybir.AluOpType.mult
    ADD = mybir.AluOpType.add
    acc = [0]

    with TileContext(nc) as tc:
        with (
            tc.tile_pool(name="vpool", bufs=1) as vpool,
            tc.tile_pool(name="stage", bufs=3) as stage,
            tc.tile_pool(name="dlt", bufs=2) as dlt,
            tc.tile_pool(name="psum", bufs=4, space=bass.MemorySpace.PSUM) as psum,
        ):
            t = vpool.tile([KI, 2, N], FP8)
            tl = vpool.tile([KI, 2, HALF], FP8)
            sc = vpool.tile([128, K * 4 + 12], F32)
            t0 = vpool.tile([6, 2, N], FP8)
            tl0 = vpool.tile([6, 2, HALF], FP8)
            ones = vpool.tile([2, 128], FP8)
            w0 = vpool.tile([128, N], FP16)
            w1 = vpool.tile([128, N], FP16)
            # (a1, b1) at base partition 0 for the level-1 replica matmul
            rep1 = vpool.tile([2, N], FP8)
            # tiny level<=2 slices land first and unblock the PE earlier
            # than the full stacks; two HWDGE rings
            nc.sync.dma_start(out=tl0[:], in_=tl_d[:6])
            nc.scalar.dma_start(out=t0[:], in_=t_d[:6])
            nc.sync.dma_start(out=rep1[:], in_=t_d[2:4, 0, :])
            nc.sync.dma_start(out=sc[:], in_=sc_d[:])
            nc.sync.dma_start(out=tl[:], in_=tl_d[:])
            nc.scalar.dma_start(out=t[:], in_=t_d[:])
            nc.gpsimd.memset(ones[:], 1.0)

            tlv = tl.rearrange("k o (m r) -> k o m r", m=128, r=4)
            tlv0 = tl0.rearrange("k o (m r) -> k o m r", m=128, r=4)

            # level 0 is rank-1: replicate v0 = a0+b0 across partitions
            # with a tiny ones-matmul (2 columns-streams instead of 8),
            # then out[p,j] = w0[j] * (v0*s0)[p] straight from SBUF
            ps0 = psum.tile([128, N], F32, tag="ps")
            for j in range(2):
                nc.tensor.matmul(
                    ps0[:, j * 512:(j + 1) * 512],
                    lhsT=ones[:, :],
                    rhs=t0[0:2, 0, j * 512:(j + 1) * 512],
                    start=True, stop=True,
                )
            nc.scalar.activation(w0[:], ps0[:], COPY)
            st0 = stage.tile([128, 4, N], I8, tag="st")
            for r in range(4):
                us_ap = sc[:, 4 * K + r:4 * K + r + 1]
                if r % 2 == 0:
                    nc.vector.tensor_scalar(st0[:, r, :], w0[:],
                                            us_ap, None, MUL)
                else:
                    nc.scalar.activation(st0[:, r, :], w0[:],
                                         COPY, scale=us_ap)
            nc.sync.dma_start(
                out=out_d[0].rearrange("(p r) f -> p r f", p=128),
                in_=st0[:, :, :],
            )

            # level 1 is rank-2: v1 replica + per-partition scaled copies
            # + GpSimd adds. Its pool ops are SPREAD one r-triple after
            # each of PE levels 2-5 (emitting them in one block ahead of
            # the loop head-of-line blocks the evacuations and stalls the
            # PE - measured +2.5us)
            ps1 = psum.tile([128, N], F32, tag="ps")
            for j in range(2):
                nc.tensor.matmul(
                    ps1[:, j * 512:(j + 1) * 512],
                    lhsT=ones[:, :],
                    rhs=rep1[:, j * 512:(j + 1) * 512],
                    start=True, stop=True,
                )
            nc.vector.tensor_copy(w1[:], ps1[:])
            st1 = stage.tile([128, 4, N], FP16, tag="st1")

            def lvl1_triple(r):
                u0_ap = sc[:, 4 * K + 4 + r:4 * K + 4 + r + 1]
                u1_ap = sc[:, 4 * K + 8 + r:4 * K + 8 + r + 1]
                d0 = dlt.tile([128, N], FP16, tag="d0")
                d1 = dlt.tile([128, N], FP16, tag="d1")
                nc.vector.tensor_scalar(d0[:], w0[:], u0_ap, None, MUL)
                nc.scalar.activation(d1[:], w1[:], COPY, scale=u1_ap)
                nc.gpsimd.tensor_tensor(st1[:, r, :], d0[:], d1[:], ADD)

            for lvl in range(2, K):
                ki = 2 * (lvl + 1)
                lhs_all, rhs_all = (tlv0, t0) if lvl < 3 else (tlv, t)
                tail = lvl == K - 1
                st = stage.tile([128, 4, N], I8, tag="st")
                for r in range(4):
                    ps = psum.tile([128, N], F32, tag="ps")  # 2 banks
                    for j in range(2):
                        nc.tensor.matmul(
                            ps[:, j * 512:(j + 1) * 512],
                            lhsT=lhs_all[:ki, :, :, r],
                            rhs=rhs_all[:ki, :, j * 512:(j + 1) * 512],
                            start=True, stop=True, perf_mode=DR,
                        )
                    s_ap = sc[:, 4 * lvl + r:4 * lvl + r + 1]
                    if tail:
                        # pipeline drain: both engines in parallel on the
                        # tile's two banks, store each r-slice immediately
                        nc.scalar.activation(st[:, r, :512], ps[:, :512],
                                             COPY, scale=s_ap)
                        nc.vector.tensor_scalar(st[:, r, 512:],
                                                ps[:, 512:], s_ap, None, MUL)
                        nc.sync.dma_start(
                            out=out_d[lvl].rearrange(
                                "(p r) f -> p r f", p=128)[:, r, :],
                            in_=st[:, r, :])
                        continue
                    # [128, 1024] scale+cast evacuation, alternating
                    # ACT:DVE ~ 8:7 (their measured op-rate ratio)
                    acc[0] += 8
                    if acc[0] >= 15:
                        acc[0] -= 15
                        nc.scalar.activation(st[:, r, :], ps[:],
                                             COPY, scale=s_ap)
                    else:
                        nc.vector.tensor_scalar(st[:, r, :],
                                                ps[:], s_ap, None, MUL)
                if not tail:
                    # all stores on the Sync ring: issuing from nc.scalar
                    # head-of-line blocks ACT's evacs behind the store's
                    # stage-tile wait and stalls the PE (measured +10us)
                    nc.sync.dma_start(
                        out=out_d[lvl].rearrange("(p r) f -> p r f", p=128),
                        in_=st[:, :, :],
                    )
                if 2 <= lvl <= 5:
                    lvl1_triple(lvl - 2)
                    if lvl == 5:
                        nc.sync.dma_start(
                            out=o1_d[:].rearrange("(p r) f -> p r f", p=128),
                            in_=st1[:, :, :],
                        )

    nc.compile()
    return nc


def _get_nc():
    global _nc_cache
    if _nc_cache is None:
        _nc_cache = _build()
    return _nc_cache


def _prepare_in_maps(evecs: np.ndarray):
    in_maps = []
    bounds = []
    for c in range(NCORES):
        b, h = divmod(c, 2)
        vt = np.ascontiguousarray(evecs[b, 0].T, dtype=np.float32)  # [K, N]
        a32 = vt.astype(FP8_NP).astype(np.float32)
        b32 = (vt - a32).astype(FP8_NP).astype(np.float32)
        sl = slice(h * HALF, (h + 1) * HALF)

        rhs = np.zeros((4 * K, N), dtype=np.float32)
        rhs[0::4] = a32
        rhs[1::4] = a32
        rhs[2::4] = b32
        lhs = np.zeros((4 * K, HALF), dtype=np.float32)
        lhs[0::4] = a32[:, sl]
        lhs[1::4] = b32[:, sl]
        lhs[2::4] = a32[:, sl]
        t = rhs.reshape(KI, 2, N).astype(FP8_NP)
        tl = lhs.reshape(KI, 2, HALF).astype(FP8_NP)

        # Cauchy-Schwarz bound -> per-(level, row) int8 scale, 2% margin
        cn = np.sqrt(np.cumsum(vt * vt, axis=0))          # [K, N]
        maxn = cn.max(axis=1)                             # [K]
        bound = cn[:, sl] * maxn[:, None] * 1.02          # [K, HALF]
        s = (127.0 / bound).astype(np.float32)
        # sc[p, 4*l + r] = s[l, 4p + r]
        sc = np.ascontiguousarray(
            s.reshape(K, 128, 4).transpose(1, 0, 2).reshape(128, K * 4)
        )
        # combined per-partition scalars for the rank-1/rank-2 direct
        # levels 0 and 1
        us0 = (vt[0, sl] * s[0]).reshape(128, 4).astype(np.float32)
        u01 = vt[0, sl].reshape(128, 4).astype(np.float32)   # unscaled
        u11 = vt[1, sl].reshape(128, 4).astype(np.float32)   # (fp16 out)
        sc = np.ascontiguousarray(np.hstack([sc, us0, u01, u11]))
        in_maps.append({"t": t, "tl": tl, "sc": sc})
        bounds.append(bound)                              # [K, HALF]
    return in_maps, bounds


def _assemble(results, bounds) -> np.ndarray:
    out = np.empty((B, K, N, N), dtype=np.float32)
    for c in range(NCORES):
        b, h = divmod(c, 2)
        q = results[c]["out"].astype(np.float32)          # [K, HALF, N]
        q *= (bounds[c] / 127.0)[:, :, None]
        out[b, :, h * HALF:(h + 1) * HALF, :] = q
        out[b, 1, h * HALF:(h + 1) * HALF, :] = results[c]["o1"]  # fp16
    return out.reshape(B, K * C, N, N)


def kernel(evecs) -> np.ndarray:
    evecs = np.asarray(evecs, dtype=np.float32)
    assert evecs.shape == (B, C, N, K), evecs.shape
    nc = _get_nc()
    in_maps, bounds = _prepare_in_maps(evecs)
    last_err = None
    for _attempt in range(3):
        try:
            r = run_bass_kernel_spmd(nc, in_maps, list(range(NCORES)))
            return _assemble(r.results, bounds)
        except Exception as e:  # transient NRT/device hiccups: retry
            last_err = e
    raise last_err


# revision 56
# speedup vs baseline: 1.2282x; 1.2282x over previous
"""Trainium2 Bass kernel for nn_ExpandEvecs.

Computes, for evecs [B=4, C=1, N=1024, K=16]:
    cube[b,l] = V[:, :l+1] @ V[:, :l+1]^T   (Gram expansion per level)
    -> [B, K, N, N] fp32 (cumsum of per-eigvec outer products over l).

Sharding: 8 cores = 4 batches x 2 row-halves; core c (b=c//2, h=c%2)
produces all 16 levels for its 512-row half. No communication.

Performance model (per core, 8.4M output elements; all rates HW-measured):
  - The PE streams one 512-column matmul per 427 ns (1.2 GHz sustained;
    the 2.4 GHz p-state needs 3 us of gapless execution, unreachable
    when PSUM drain paces the PE) -> 128 matmuls = 54.7 us. This is the
    kernel's floor: the PE is the only engine that can produce outer
    products at rate (GpSimd tensor ops measured 2.1-15 us per 131K
    elems, DVE fused STT 9.4 us -> offload designs all lose).
  - PSUM evacuation: only ACT (1.2 GHz) and DVE (0.96 GHz) have PSUM
    ports. Whole [128, 2048] ops (one 4-bank PSUM tile) alternate
    between them ~53:47 -> ~35 us in parallel, under the PE floor.
  - int8 output (8.4 MB -> ~23.5 us at the ~358 GB/s per-core HBM
    limit) keeps DMA far off the critical path; fp32 would be 94 us.

Precision (gate 2e-2; simulated end-to-end 4.5e-3):
  - fp8 split matmuls: V = A + B, A = e4m3(V), B = e4m3(V-A);
    V V^T ~= A A^T + A B^T + B A^T (dropped B B^T ~2^-8). 4 stacked
    rows per eigvec k (lhs A,B,A,0 / rhs A,A,B,0) so DoubleRow pairs
    never straddle a k boundary.
  - int8 scale per (level, partition) from the Cauchy-Schwarz bound
    max over the partition's 4 interleaved rows of
    ||v_i||_l * max_j ||v_j||_l (host-computed, 2% margin), applied
    during evacuation (ACT activation scale= / DVE tensor_scalar,
    which round to nearest). Host dequantizes during the unshard.
  - Row-pair interleave: partition p holds DRAM rows 4p..4p+3, giving
    4 KiB contiguous int8 store runs per partition.
"""

import numpy as np
import ml_dtypes

import concourse.mybir as mybir
from concourse import bacc, bass
from concourse.tile import TileContext
from concourse.bass_utils import run_bass_kernel_spmd

B, C, N, K = 4, 1, 1024, 16
NCORES = 8
HALF = N // 2          # rows per core
KI = 2 * K             # DoubleRow pair-partitions at the deepest level

F32 = mybir.dt.float32
FP16 = mybir.dt.float16
FP8 = mybir.dt.float8e4
I8 = mybir.dt.int8
FP8_NP = ml_dtypes.float8_e4m3

_nc_cache = None


def _build():
    nc = bacc.Bacc(None, target_bir_lowering=False)
    t_d = nc.declare_dram_parameter("t", [KI, 2, N], FP8, isOutput=False)
    tl_d = nc.declare_dram_parameter("tl", [KI, 2, HALF], FP8, isOutput=False)
    # sc columns [0, 4K) = per-(level,row) int8 scales; [4K, 4K+4) =
    # v0*s0 for the rank-1 direct level 0; [4K+4, 4K+8) = v0*s1 and
    # [4K+8, 4K+12) = v1*s1 for the rank-2 direct level 1
    sc_d = nc.declare_dram_parameter("sc", [128, K * 4 + 12], F32, isOutput=False)
    out_d = nc.declare_dram_parameter("out", [K, HALF, N], I8, isOutput=True)
    # level 1 ships as unscaled fp16: its rank-2 direct path adds on
    # GpSimd, whose tensor_tensor only codegens with fp16 output
    o1_d = nc.declare_dram_parameter("o1", [HALF, N], FP16, isOutput=True)

    DR = mybir.MatmulPerfMode.DoubleRow
    COPY = mybir.ActivationFunctionType.Copy
    MUL = mybir.AluOpType.mult
    ADD = mybir.AluOpType.add
    acc = [0]

    with TileContext(nc) as tc:
        with (
            tc.tile_pool(name="vpool", bufs=1) as vpool,
            tc.tile_pool(name="stage", bufs=3) as stage,
            tc.tile_pool(name="dlt", bufs=2) as dlt,
            tc.tile_pool(name="psum", bufs=4, space=bass.MemorySpace.PSUM) as psum,
        ):
            t = vpool.tile([KI, 2, N], FP8)
            tl = vpool.tile([KI, 2, HALF], FP8)
            sc = vpool.tile([128, K * 4 + 12], F32)
            t0 = vpool.tile([6, 2, N], FP8)
            tl0 = vpool.tile([6, 2, HALF], FP8)
            ones = vpool.tile([2, 128], FP8)
            w0 = vpool.tile([128, N], FP16)
            w1 = vpool.tile([128, N], FP16)
            # (a1, b1) at base partition 0 for the level-1 replica matmul
            rep1 = vpool.tile([2, N], FP8)
            # tiny level<=2 slices land first and unblock the PE earlier
            # than the full stacks; two HWDGE rings
            nc.sync.dma_start(out=tl0[:], in_=tl_d[:6])
            nc.scalar.dma_start(out=t0[:], in_=t_d[:6])
            nc.sync.dma_start(out=rep1[:], in_=t_d[2:4, 0, :])
            nc.sync.dma_start(out=sc[:], in_=sc_d[:])
            nc.sync.dma_start(out=tl[:], in_=tl_d[:])
            nc.scalar.dma_start(out=t[:], in_=t_d[:])
            nc.gpsimd.memset(ones[:], 1.0)

            tlv = tl.rearrange("k o (m r) -> k o m r", m=128, r=4)
            tlv0 = tl0.rearrange("k o (m r) -> k o m r", m=128, r=4)

            # level 0 is rank-1: replicate v0 = a0+b0 across partitions
            # with a tiny ones-matmul (2 columns-streams instead of 8),
            # then out[p,j] = w0[j] * (v0*s0)[p] straight from SBUF
            ps0 = psum.tile([128, N], F32, tag="ps")
            for j in range(2):
                nc.tensor.matmul(
                    ps0[:, j * 512:(j + 1) * 512],
                    lhsT=ones[:, :],
                    rhs=t0[0:2, 0, j * 512:(j + 1) * 512],
                    start=True, stop=True,
                )
            nc.scalar.activation(w0[:], ps0[:], COPY)
            st0 = stage.tile([128, 4, N], I8, tag="st")
            for r in range(4):
                us_ap = sc[:, 4 * K + r:4 * K + r + 1]
                if r % 2 == 0:
                    nc.vector.tensor_scalar(st0[:, r, :], w0[:],
                                            us_ap, None, MUL)
                else:
                    nc.scalar.activation(st0[:, r, :], w0[:],
                                         COPY, scale=us_ap)
            nc.sync.dma_start(
                out=out_d[0].rearrange("(p r) f -> p r f", p=128),
                in_=st0[:, :, :],
            )

            # level 1 is rank-2: v1 replica + per-partition scaled copies
            # + GpSimd adds. Its pool ops are SPREAD one r-triple after
            # each of PE levels 2-5 (emitting them in one block ahead of
            # the loop head-of-line blocks the evacuations and stalls the
            # PE - measured +2.5us)
            ps1 = psum.tile([128, N], F32, tag="ps")
            for j in range(2):
                nc.tensor.matmul(
                    ps1[:, j * 512:(j + 1) * 512],
                    lhsT=ones[:, :],
                    rhs=rep1[:, j * 512:(j + 1) * 512],
                    start=True, stop=True,
                )
            nc.vector.tensor_copy(w1[:], ps1[:])
            st1 = stage.tile([128, 4, N], FP16, tag="st1")

            def lvl1_triple(r):
                u0_ap = sc[:, 4 * K + 4 + r:4 * K + 4 + r + 1]
                u1_ap = sc[:, 4 * K + 8 + r:4 * K + 8 + r + 1]
                d0 = dlt.tile([128, N], FP16, tag="d0")
                d1 = dlt.tile([128, N], FP16, tag="d1")
                nc.vector.tensor_scalar(d0[:], w0[:], u0_ap, None, MUL)
                nc.scalar.activation(d1[:], w1[:], COPY, scale=u1_ap)
                nc.gpsimd.tensor_tensor(st1[:, r, :], d0[:], d1[:], ADD)

            for lvl in range(2, K):
                ki = 2 * (lvl + 1)
                lhs_all, rhs_all = (tlv0, t0) if lvl < 3 else (tlv, t)
                tail = lvl == K - 1
                st = stage.tile([128, 4, N], I8, tag="st")
                for r in range(4):
                    ps = psum.tile([128, N], F32, tag="ps")  # 2 banks
                    for j in range(2):
                        nc.tensor.matmul(
                            ps[:, j * 512:(j + 1) * 512],
                            lhsT=lhs_all[:ki, :, :, r],
                            rhs=rhs_all[:ki, :, j * 512:(j + 1) * 512],
                            start=True, stop=True, perf_mode=DR,
                        )
                    s_ap = sc[:, 4 * lvl + r:4 * lvl + r + 1]
                    if tail:
                        # pipeline drain: both engines in parallel on the
                        # tile's two banks, store each r-slice immediately
                        nc.scalar.activation(st[:, r, :512], ps[:, :512],
                                             COPY, scale=s_ap)
                        nc.vector.tensor_scalar(st[:, r, 512:],
                                                ps[:, 512:], s_ap, None, MUL)
                        nc.sync.dma_start(
                            out=out_d[lvl].rearrange(
                                "(p r) f -> p r f", p=128)[:, r, :],
                            in_=st[:, r, :])
                        continue
                    # [128, 1024] scale+cast evacuation, alternating
                    # ACT:DVE ~ 8:7 (their measured op-rate ratio)
                    acc[0] += 8
                    if acc[0] >= 15:
                        acc[0] -= 15
                        nc.scalar.activation(st[:, r, :], ps[:],
                                             COPY, scale=s_ap)
                    else:
                        nc.vector.tensor_scalar(st[:, r, :],
                                                ps[:], s_ap, None, MUL)
                if not tail:
                    # all stores on the Sync ring: issuing from nc.scalar
                    # head-of-line blocks ACT's evacs behind the store's
                    # stage-tile wait and stalls the PE (measured +10us)
                    nc.sync.dma_start(
                        out=out_d[lvl].rearrange("(p r) f -> p r f", p=128),
                        in_=st[:, :, :],
                    )
                if lvl in (3, 6, 9, 12):
                    # one triple per 3 PE levels: ~2us of pool slack
                    # absorbs each 2.5us insertion without stalling the PE
                    lvl1_triple((lvl - 3) // 3)
                    if lvl == 12:
                        nc.sync.dma_start(
                            out=o1_d[:].rearrange("(p r) f -> p r f", p=128),
                            in_=st1[:, :, :],
                        )

    nc.compile()
    return nc


def _get_nc():
    global _nc_cache
    if _nc_cache is None:
        _nc_cache = _build()
    return _nc_cache


def _prepare_in_maps(evecs: np.ndarray):
    in_maps = []
    bounds = []
    for c in range(NCORES):
        b, h = divmod(c, 2)
        vt = np.ascontiguousarray(evecs[b, 0].T, dtype=np.float32)  # [K, N]
        a32 = vt.astype(FP8_NP).astype(np.float32)
        b32 = (vt - a32).astype(FP8_NP).astype(np.float32)
        sl = slice(h * HALF, (h + 1) * HALF)

        rhs = np.zeros((4 * K, N), dtype=np.float32)
        rhs[0::4] = a32
        rhs[1::4] = a32
        rhs[2::4] = b32
        lhs = np.zeros((4 * K, HALF), dtype=np.float32)
        lhs[0::4] = a32[:, sl]
        lhs[1::4] = b32[:, sl]
        lhs[2::4] = a32[:, sl]
        t = rhs.reshape(KI, 2, N).astype(FP8_NP)
        tl = lhs.reshape(KI, 2, HALF).astype(FP8_NP)

        # Cauchy-Schwarz bound -> per-(level, row) int8 scale, 2% margin
        cn = np.sqrt(np.cumsum(vt * vt, axis=0))          # [K, N]
        maxn = cn.max(axis=1)                             # [K]
        bound = cn[:, sl] * maxn[:, None] * 1.02          # [K, HALF]
        s = (127.0 / bound).astype(np.float32)
        # sc[p, 4*l + r] = s[l, 4p + r]
        sc = np.ascontiguousarray(
            s.reshape(K, 128, 4).transpose(1, 0, 2).reshape(128, K * 4)
        )
        # combined per-partition scalars for the rank-1/rank-2 direct
        # levels 0 and 1
        us0 = (vt[0, sl] * s[0]).reshape(128, 4).astype(np.float32)
        u01 = vt[0, sl].reshape(128, 4).astype(np.float32)   # unscaled
        u11 = vt[1, sl].reshape(128, 4).astype(np.float32)   # (fp16 out)
        sc = np.ascontiguousarray(np.hstack([sc, us0, u01, u11]))
        in_maps.append({"t": t, "tl": tl, "sc": sc})
        bounds.append(bound)                              # [K, HALF]
    return in_maps, bounds


def _assemble(results, bounds) -> np.ndarray:
    out = np.empty((B, K, N, N), dtype=np.float32)
    for c in range(NCORES):
        b, h = divmod(c, 2)
        q = results[c]["out"].astype(np.float32)          # [K, HALF, N]
        q *= (bounds[c] / 127.0)[:, :, None]
        out[b, :, h * HALF:(h + 1) * HALF, :] = q
        out[b, 1, h * HALF:(h + 1) * HALF, :] = results[c]["o1"]  # fp16
    return out.reshape(B, K * C, N, N)


def kernel(evecs) -> np.ndarray:
    evecs = np.asarray(evecs, dtype=np.float32)
    assert evecs.shape == (B, C, N, K), evecs.shape
    nc = _get_nc()
    in_maps, bounds = _prepare_in_maps(evecs)
    last_err = None
    for _attempt in range(3):
        try:
            r = run_bass_kernel_spmd(nc, in_maps, list(range(NCORES)))
            return _assemble(r.results, bounds)
        except Exception as e:  # transient NRT/device hiccups: retry
            last_err = e
    raise last_err
